# revision 93
# baseline (speedup 1.0000x reference)
import os
import sys

sys.path.insert(0, "/opt/trn_rl_repo")

import numpy as np
import ml_dtypes

BF16NP = ml_dtypes.bfloat16

import concourse.bacc as bacc
import concourse.mybir as mybir
from concourse.tile import TileContext
from concourse.bass_utils import run_bass_kernel_spmd

F32 = mybir.dt.float32
F32R = mybir.dt.float32r
BF16 = mybir.dt.bfloat16
AF = mybir.ActivationFunctionType
ALU = mybir.AluOpType

B, S, D, H = 2, 2048, 1024, 16
HD = D // H
NCORES = 8
HPC = 4
E = HPC * HD
EV = HPC * (HD + 1)
DCH = D // 128
SKC = S // 128
SCALE = 1.0 / np.sqrt(HD)
FE_A = float(2 ** 23 / np.log(2)) * float(SCALE)
FE_B = float(127 * 2 ** 23 - 486411)
FE_MOD = int(os.environ.get("FE_MOD", "0"))
FE_START = int(os.environ.get("FE_START", "8"))
FE_SLOTS = {int(t) for t in os.environ.get("FE_SLOTS", "1,4,6").split(",") if t}
FE_POOL = os.environ.get("FE_POOL", "0") == "1"
FE_ALT = {int(t) for t in os.environ.get("FE_ALT", "4").split(",") if t}

PACE_NUM = int(os.environ.get("PACE_NUM", "1"))
PACE_DEN = int(os.environ.get("PACE_DEN", "4"))
PIPE_DEPTH = int(os.environ.get("PIPE_DEPTH", "3"))


def build_kernel(repeat: int = 1, loop_n: int = 1):
    nc = bacc.Bacc()
    xqT = nc.dram_tensor("xqT", [D, S], BF16, kind="ExternalInput")
    xkT = nc.dram_tensor("xkT", [D, S], BF16, kind="ExternalInput")
    xvT = nc.dram_tensor("xvT", [D, S], BF16, kind="ExternalInput")
    wq = nc.dram_tensor("wq", [D, E], BF16, kind="ExternalInput")
    wk = nc.dram_tensor("wk", [D, E], BF16, kind="ExternalInput")
    wv = nc.dram_tensor("wv", [D, EV], BF16, kind="ExternalInput")
    bq = nc.dram_tensor("bq", [128, 2], F32, kind="ExternalInput")
    bk = nc.dram_tensor("bk", [128, 2], F32, kind="ExternalInput")
    bv = nc.dram_tensor("bv", [128, EV], F32, kind="ExternalInput")
    out = nc.dram_tensor("out", [S, HPC * (HD + 1)], F32, kind="ExternalOutput")

    with TileContext(nc) as tc:
        with tc.tile_pool(name="wsb", bufs=1) as wsb, \
             tc.tile_pool(name="xsb", bufs=12) as xsb, \
             tc.tile_pool(name="qkv", bufs=1) as qkv, \
             tc.tile_pool(name="esb", bufs=10) as esb, \
             tc.tile_pool(name="osb", bufs=4) as osb, \
             tc.tile_pool(name="pps", bufs=2, space="PSUM") as pps, \
             tc.tile_pool(name="stp", bufs=2, space="PSUM") as stp, \
             tc.tile_pool(name="pvp", bufs=2, space="PSUM") as pvp:

            wk_t = wsb.tile([128, DCH, E], BF16)
            wkr = wk.rearrange("(c p) e -> p c e", p=128)
            nc.sync.dma_start(wk_t[:, 0:4], wkr[:, 0:4])
            bk_t = wsb.tile([128, 2], F32)
            nc.sync.dma_start(bk_t[:], bk[:])
            warm = wsb.tile([128, 1], F32)
            nc.scalar.activation(warm[:], bk_t[:, 0:1], AF.Exp)

            def load_wq():
                wq_t = wsb.tile([128, DCH, E], BF16, name="wq_t")
                nc.sync.dma_start(wq_t[:], wq.rearrange("(c p) e -> p c e", p=128))
                bq_t = wsb.tile([128, 2], F32, name="bq_t")
                nc.sync.dma_start(bq_t[:], bq[:])
                return wq_t, bq_t

            def load_wv():
                wv_t = wsb.tile([128, DCH, EV], BF16, name="wv_t")
                nc.sync.dma_start(wv_t[:], wv.rearrange("(c p) e -> p c e", p=128))
                bv_t = wsb.tile([128, EV], F32, name="bv_t")
                nc.sync.dma_start(bv_t[:], bv[:])
                return wv_t, bv_t

            def load_x(src, si, chunked=False, defer_half=False):
                sl = slice(512 * si, 512 * (si + 1))
                x_t = xsb.tile([128, DCH, 512], BF16, tag="x", name=f"x_{si}")
                xr = src[:, sl].rearrange("(c p) s -> p c s", p=128)
                if defer_half:
                    nc.sync.dma_start(x_t[:, :, 0:256], xr[:, :, 0:256])
                    return x_t, lambda: nc.sync.dma_start(
                        x_t[:, :, 256:512], xr[:, :, 256:512])
                if chunked:
                    nc.sync.dma_start(x_t[:, 0:4], xr[:, 0:4])
                    nc.sync.dma_start(x_t[:, 4:8], xr[:, 4:8])
                else:
                    nc.sync.dma_start(x_t[:], xr)
                return x_t

            import contextlib

            def body_scope():
                if loop_n > 1:
                    return tc.For_i(0, loop_n, 1)
                return contextlib.nullcontext()

            for _ in range(repeat):
              with body_scope():
                QT_t = qkv.tile([128, 2, S], BF16, tag="QT", name="QT_t")
                KT_t = qkv.tile([128, 4, S], BF16, tag="KT", name="KT_t")
                V_t = qkv.tile([128, SKC, EV], BF16, tag="V", name="V_t")

                def proj_qk_group(x_t, w_t, b_t, o_t, si, et, kt):
                    sl = slice(512 * si, 512 * (si + 1))
                    ps = pps.tile([128, 512], F32, tag="pj", name="ps_qk")
                    for c in range(DCH):
                        nc.tensor.matmul(
                            ps[:], w_t[:, c, 128 * et:128 * (et + 1)],
                            x_t[:, c], start=(c == 0), stop=(c == DCH - 1))
                    if kt:
                        nc.vector.tensor_scalar(
                            out=o_t[0:64, 2 * et, sl], in0=ps[0:64, :],
                            scalar1=b_t[0:64, et:et + 1], scalar2=None,
                            op0=ALU.add)
                        nc.vector.tensor_scalar(
                            out=o_t[64:128, 2 * et + 1, sl], in0=ps[64:128, :],
                            scalar1=b_t[64:128, et:et + 1], scalar2=None,
                            op0=ALU.add)
                    else:
                        nc.vector.tensor_scalar(
                            out=o_t[:, et, sl], in0=ps[:],
                            scalar1=b_t[:, et:et + 1], scalar2=None,
                            op0=ALU.add)

                def proj_v_group(x_t, si, k):
                    psv = pps.tile([128, EV], F32, tag="pj", name="ps_v")
                    for c in range(DCH):
                        nc.tensor.matmul(
                            psv[:], x_t[:, c, 128 * k:128 * (k + 1)],
                            wv_t[:, c], start=(c == 0), stop=(c == DCH - 1))
                    nc.vector.tensor_tensor(
                        out=V_t[:, 4 * si + k, :], in0=psv[:], in1=bv_t[:],
                        op=ALU.add)

                for _h in range(4):
                    _lo, _hi = (64, 128) if _h % 2 == 0 else (0, 64)
                    eng = nc.gpsimd if _h % 2 == 0 else nc.vector
                    eng.memset(KT_t[_lo:_hi, _h, :], 0.0)

                x_state = {}

                def ensure_x(kind, si):
                    key = (kind, si)
                    if key not in x_state:
                        src = {"k": xkT, "q": xqT, "v": xvT}[kind]
                        x_state[key] = load_x(src, si, chunked=(key in (("k", 0), ("q", 0), ("k", 1), ("q", 1), ("k", 2), ("v", 2), ("k", 3), ("q", 2), ("v", 3))))
                    return x_state[key]

                emitted = set()
                state = {"cpl": 0, "fill": 0}
                queue = []

                def emit_group(tag):
                    if tag in emitted:
                        return
                    emitted.add(tag)
                    kind = tag[0]
                    if kind == "K":
                        _, si, et = tag
                        proj_qk_group(ensure_x("k", si), wk_t, bk_t, KT_t,
                                      si, et, kt=True)
                    elif kind == "Q":
                        _, sqt, pr = tag
                        proj_qk_group(ensure_x("q", sqt), wq_t, bq_t, QT_t,
                                      sqt, pr, kt=False)
                    else:
                        _, si, k = tag
                        proj_v_group(ensure_x("v", si), si, k)
                    state["fill"] += 1

                def need(*tags):
                    for t in tags:
                        emit_group(t)

                def pace_fill():
                    state["cpl"] += 1
                    while queue and queue[0] in emitted:
                        queue.pop(0)
                    if queue and state["fill"] * PACE_DEN <= state["cpl"] * PACE_NUM:
                        emit_group(queue.pop(0))

                ovs = {}

                def attention_piece(pr, sqt, g, last=False):
                    sq0 = 512 * sqt
                    need(("Q", sqt, pr))
                    pvh = [pvp.tile([128, 4, 80], F32, tag="pv", bufs=2,
                                    name=f"pv{h}") for h in range(2)]

                    def emit_pv(ets, cks, cpl):
                        for ck in cks:
                            need(("V", ck // 4, ck % 4))
                        for h in range(2):
                            hh = 2 * pr + h
                            for q in range(2):
                                for j in range(4):
                                    nc.tensor.matmul(
                                        pvh[h][:, j, 0:65],
                                        ets[h][:, 512 * q + 128 * j:
                                               512 * q + 128 * (j + 1)],
                                        V_t[:, cks[q], 65 * hh:65 * hh + 65],
                                        start=(cpl == 0 and q == 0
                                               and j == 0),
                                        stop=(cpl == 3 and q == 1
                                              and j == 3))

                    pipeq = []
                    for cpl in range(4):
                        cks = (8 * g + 2 * cpl, 8 * g + 2 * cpl + 1)
                        for ck in cks:
                            need(("K", ck // 4, pr))
                        sts = []
                        for h in range(2):
                            st = stp.tile([128, 1024], F32, tag="st",
                                          name=f"st{h}")
                            sts.append(st)
                        for q in range(2):
                            for h in range(2):
                                hh = 2 * pr + h
                                ck = cks[q]
                                nc.tensor.matmul(
                                    sts[h][:, 512 * q:512 * (q + 1)],
                                    KT_t[:, hh, 128 * ck:128 * (ck + 1)],
                                    QT_t[:, pr, sq0:sq0 + 512],
                                    start=True, stop=True)
                        ets = []
                        for h in range(2):
                            e_t = esb.tile([128, 1024], BF16, name="e_t")
                            _slot = 2 * cpl + h
                            _sl = FE_SLOTS | FE_ALT if pidx % 2 else FE_SLOTS
                            _off = (_sl and _slot in _sl) or (
                                FE_MOD and _slot % FE_MOD == FE_MOD - 1
                                and not _sl)
                            if _off and pidx >= FE_START:
                                ti = esb.tile([128, 1024], mybir.dt.int32,
                                              tag="ti", bufs=4, name="ti")
                                nc.vector.tensor_scalar(
                                    out=ti[:], in0=sts[h][:], scalar1=FE_A,
                                    scalar2=FE_B, op0=ALU.mult, op1=ALU.add)
                                ceng = nc.gpsimd if FE_POOL else nc.vector
                                ceng.tensor_copy(e_t[:],
                                                 ti[:].bitcast(F32))
                            else:
                                nc.scalar.activation(e_t[:], sts[h][:],
                                                     AF.Exp,
                                                     scale=float(SCALE))
                            ets.append(e_t)
                        pace_fill()
                        pipeq.append((ets, cks, cpl))
                        depth = 1 if last else PIPE_DEPTH
                        if len(pipeq) > depth:
                            emit_pv(*pipeq.pop(0))
                    while pipeq:
                        emit_pv(*pipeq.pop(0))
                    for h in range(2):
                        hh = 2 * pr + h
                        dst = out[sq0:sq0 + 512, 65 * hh:65 * (hh + 1)]
                        dst = dst.rearrange("(j p) e -> p j e", p=128)
                        if g == 0:
                            ov = osb.tile([128, 4, 65], F32, tag="ov", bufs=14,
                                          name=f"ov{pr}{sqt}{h}")
                            ovs[(pr, sqt, h)] = ov
                            nc.vector.tensor_copy(ov[:], pvh[h][:, :, 0:65])
                        elif not last:
                            ov = ovs.pop((pr, sqt, h))
                            nc.vector.tensor_tensor(
                                out=ov[:], in0=ov[:], in1=pvh[h][:, :, 0:65],
                                op=ALU.add)
                            nc.gpsimd.dma_start(dst, ov[:])
                        else:
                            ov = ovs.pop((pr, sqt, h))
                            nc.vector.tensor_tensor(
                                out=ov[:], in0=ov[:], in1=pvh[h][:, :, 0:65],
                                op=ALU.add)
                            nc.sync.dma_start(dst, ov[:])

                nc.sync.dma_start(wk_t[:, 4:8], wkr[:, 4:8])
                need(("K", 0, 0), ("K", 0, 1))
                wq_t, bq_t = load_wq()
                need(("Q", 0, 0), ("Q", 0, 1))
                wv_t, bv_t = load_wv()
                need(("V", 0, 0), ("V", 0, 1), ("V", 0, 2), ("V", 0, 3))
                need(("K", 1, 0), ("K", 1, 1))
                need(("V", 1, 0), ("V", 1, 1), ("V", 1, 2), ("V", 1, 3))
                state["fill"] = 0

                queue.extend([
                    ("Q", 1, 0), ("Q", 1, 1),
                    ("K", 2, 0), ("K", 2, 1),
                    ("V", 2, 0), ("V", 2, 1), ("V", 2, 2), ("V", 2, 3),
                    ("K", 3, 0), ("K", 3, 1),
                    ("V", 3, 0), ("V", 3, 1), ("V", 3, 2), ("V", 3, 3),
                    ("Q", 2, 0), ("Q", 2, 1),
                    ("Q", 3, 0), ("Q", 3, 1),
                ])

                for i, (pr, sqt, g) in enumerate(PIECE_ORDER):
                    attention_piece(pr, sqt, g, last=(i == len(PIECE_ORDER) - 1))
    nc.compile()
    return nc


_NC_CACHE = {}


def _get_nc(repeat: int = 1, loop_n: int = 1):
    key = (repeat, loop_n)
    if key not in _NC_CACHE:
        _NC_CACHE[key] = build_kernel(repeat, loop_n)
    return _NC_CACHE[key]


def _shard_inputs(q, k, v, Wq, bq, Wk, bk, Wv, bv):
    xT = {}
    for b in range(B):
        xT[("q", b)] = np.ascontiguousarray(np.asarray(q)[b].T).astype(BF16NP)
        xT[("k", b)] = np.ascontiguousarray(np.asarray(k)[b].T).astype(BF16NP)
        xT[("v", b)] = np.ascontiguousarray(np.asarray(v)[b].T).astype(BF16NP)
    Wq, Wk, Wv = (np.asarray(a, np.float32) for a in (Wq, Wk, Wv))
    bq, bk, bv = (np.asarray(a, np.float32) for a in (bq, bk, bv))
    in_maps = []
    for c in range(NCORES):
        b, g = divmod(c, HPC)
        sl = slice(E * g, E * (g + 1))
        wv_p = np.zeros((D, EV), np.float32)
        bv_p = np.zeros((128, EV), np.float32)
        for h in range(HPC):
            wv_p[:, 65 * h:65 * h + HD] = Wv[:, E * g + HD * h:E * g + HD * (h + 1)]
            bv_p[:, 65 * h:65 * h + HD] = bv[E * g + HD * h:E * g + HD * (h + 1)]
            bv_p[:, 65 * h + HD] = 1.0
        in_maps.append({
            "xqT": xT[("q", b)], "xkT": xT[("k", b)], "xvT": xT[("v", b)],
            "wq": np.ascontiguousarray(Wq[:, sl]).astype(BF16NP),
            "wk": np.ascontiguousarray(Wk[:, sl]).astype(BF16NP),
            "wv": wv_p.astype(BF16NP),
            "bq": np.ascontiguousarray(bq[sl].reshape(2, 128).T),
            "bk": np.ascontiguousarray(bk[sl].reshape(2, 128).T),
            "bv": bv_p,
        })
    return in_maps


def kernel(q, k, v, Wq, bq, Wk, bk, Wv, bv):
    nc = _get_nc()
    in_maps = _shard_inputs(q, k, v, Wq, bq, Wk, bk, Wv, bv)
    res = run_bass_kernel_spmd(nc, in_maps, core_ids=list(range(NCORES)))
    outp = np.empty((B, S, D), np.float32)
    for c in range(NCORES):
        b, g = divmod(c, HPC)
        o = res.results[c]["out"]
        for h in range(HPC):
            blk = o[:, 65 * h:65 * h + HD]
            den = o[:, 65 * h + HD:65 * h + HD + 1]
            outp[b, :, E * g + HD * h:E * g + HD * (h + 1)] = blk / den
    return outp
